# revision 18
# baseline (speedup 1.0000x reference)
"""Trainium2 Bass kernel for nn_CustomRetrieverModel (retrieval_knn).

Late-interaction retriever scoring:
  sim4d = l2n(q_tok) @ l2n(d_tok * punct).T  -> max over doc tokens
  -> valid-weighted mean over query tokens -> avg_sim (B, M)
  logits = shuffle(avg_sim) * shuffle(Wq) * exp(log_inv_t)
  with Wq from L2-normalized CLS vectors: (center - min cand)/2.

Sharding: data-parallel over the M (document) axis. Each of the 8 cores
scores all B=32 queries against M/8 = 8 docs; q replicated, host
concatenates the per-core (B, 8) logits and applies the even/odd column
shuffle (a pure output permutation commutes with the elementwise finale).

v2 device plan (fp8 DoubleRow main matmul, no device transposes):
  - host pre-transposes q and d into the PE DoubleRow fp8 layout
    [128p, kk, i, n] with h = kk*256 + i*128 + p (raw values, e4m3).
  - q is NOT normalized on device: max over doc tokens commutes with the
    positive row scale 1/||q||, folded into the weighted-sum weights
    (q_valid/||q||, from a separate bf16 q_rows Square pass).
  - d normalization = per-COLUMN scale of dT: Square(fp8 dT) -> fp8,
    ones-weights DoubleRow matmul gives sumsq broadcast over all 128
    partitions in PSUM; scalar sqrt(+eps) + DVE reciprocal + punct/pad
    column mask -> s; DVE multiplies the fp8 dT columns in place.
  - main matmul: 3 DoubleRow fp8 accums per (128 q rows x 512 d tokens)
    PSUM tile; reduce_max over each doc's 256 columns split across DVE
    and GpSimd; per-q-tile weighted-sum matmul (bf16 W block-diagonal)
    accumulates the (32, 8) sum_sim in PSUM.
  - pad d tokens are zeroed (not -1e-9-masked): only changes the max
    when every real token sims below -1e-9, an O(1e-9) absolute effect.
"""

import sys

for _p in ("/opt/trn_rl_repo",):
    if _p not in sys.path:
        sys.path.append(_p)

import math

import numpy as np
import ml_dtypes

import concourse.bass as bass
import concourse.tile as tile
from concourse import bacc, mybir
import concourse.bass_utils as bass_utils

# ---- problem shape (hardcoded per spec) ----
B, LQ, M, LD, H, L = 32, 64, 64, 256, 768, 3
NCORES = 8
MLOC = M // NCORES          # 8 docs per core
BQ = B * LQ                 # 2048 query rows
DR = MLOC * LD              # 2048 doc-token rows per core
KK = H // 256               # 3 DoubleRow contraction chunks (256 K each)
QT = BQ // 128              # 16 q row tiles
NCG = DR // 512             # 4 doc-column groups (512 tokens = 2 docs)

EPS_NORM = 1e-12
EPS_DIV = 1e-10
LN2 = math.log(2.0)

F32 = mybir.dt.float32
BF16 = mybir.dt.bfloat16
F8 = mybir.dt.float8e4
I32 = mybir.dt.int32
PERF2 = mybir.MatmulPerfMode.DoubleRow

NP_F8 = ml_dtypes.float8_e4m3
NP_BF16 = ml_dtypes.bfloat16


def _emit(nc, tc, io):
    qt8_d = io["qt8"].ap()        # (4*128, KK*2*512) f8  q^T qc-major
    dt8_d = io["dt8"].ap()        # (NCG*128, KK*2*512) f8  d^T cg-major
    q_rows = io["q_rows"].ap()    # (BQ, H) bf16
    qids = io["qids"].ap()        # (32, 64) i32
    qcls = io["qcls"].ap()        # (32, 768) f32   q_cls[-1]
    dcls = io["dcls"].ap()        # (24, 768) f32   d_cls shard (l*8+m, h)
    logt = io["logt"].ap()        # (32, 1)  f32
    out = io["out"].ap()          # (32, 8)  f32

    AF = mybir.ActivationFunctionType
    ALU = mybir.AluOpType

    import contextlib
    ctx = contextlib.ExitStack()
    singles = ctx.enter_context(tc.tile_pool(name="singles", bufs=1))
    smalls = ctx.enter_context(tc.tile_pool(name="smalls", bufs=4))
    clsbig = ctx.enter_context(tc.tile_pool(name="clsbig", bufs=1))

    # ---------- big operand tiles ----------
    qT8 = singles.tile([128, 4, KK, 2, 512], F8)
    dT8 = singles.tile([128, NCG, KK, 2, 512], F8)
    ones8 = singles.tile([128, 2, 128], F8)
    ident = singles.tile([128, 128], F32)

    # kick the big DMAs in priority order on ONE ring (they stream in
    # sequence at full fan-out): tiny id/scalar tensors first (they gate
    # early DVE/scalar setup), then d cg0 (normalization chain), qT8
    # (first main matmul), cg1-3. dmask rides the gpsimd ring.
    nc.sync.dma_start(dT8[:, 0], dt8_d[0:128, :])
    nc.sync.dma_start(qT8[:, 0], qt8_d[0:128, :])
    nc.sync.dma_start(qT8[:, 1], qt8_d[128:256, :])
    nc.sync.dma_start(dT8[:, 1], dt8_d[128:256, :])
    nc.sync.dma_start(qT8[:, 2], qt8_d[256:384, :])
    nc.sync.dma_start(qT8[:, 3], qt8_d[384:512, :])
    nc.sync.dma_start(dT8[:, 2], dt8_d[256:384, :])
    nc.sync.dma_start(dT8[:, 3], dt8_d[384:512, :])

    nc.vector.memset(ones8, 1.0)
    nc.gpsimd.memset(ident, 1.0)
    nc.gpsimd.affine_select(
        out=ident, in_=ident, pattern=[[-1, 128]], base=0,
        channel_multiplier=1, compare_op=ALU.is_equal, fill=0.0,
    )

    # ---------- q-id masks ----------
    # q_ids in per-tile layout (host pre-transposed): tile[p, c] = ids[c*128+p]
    qid_t = singles.tile([128, QT], I32)
    with tc.high_priority():
        nc.sync.dma_start(qid_t, io["qids_t"].ap())
    qv = singles.tile([128, QT], F32)       # 1.0 where q_ids != 0
    nc.vector.tensor_scalar(qv, qid_t, 0.0, None, op0=ALU.is_equal)
    nc.vector.tensor_scalar(qv, qv, -1.0, 1.0, op0=ALU.mult, op1=ALU.add)

    # n_valid from the natural (32, 64) layout: 64 - sum(q_ids == 0)
    qid_n = smalls.tile([32, 64], I32)
    with tc.high_priority():
        nc.sync.dma_start(qid_n, qids)
    qv_n = smalls.tile([32, 64], F32)
    nc.vector.tensor_scalar(qv_n, qid_n, 0.0, None, op0=ALU.is_equal)
    nv_eq = smalls.tile([32, 1], F32)
    nc.vector.reduce_sum(nv_eq, qv_n, axis=mybir.AxisListType.X)
    n_valid = smalls.tile([32, 1], F32)     # 64 - sum(eq) + eps
    nc.vector.tensor_scalar(n_valid, nv_eq, -1.0, 64.0 + EPS_DIV,
                            op0=ALU.mult, op1=ALU.add)
    rnv = smalls.tile([32, 1], F32)
    nc.vector.reciprocal(rnv, n_valid)

    lt_t = smalls.tile([32, 1], F32)
    it_half = smalls.tile([32, 1], F32, tag="ith")
    bln2 = smalls.tile([32, 1], F32, tag="bln2")
    seed = singles.tile([128, 1], F32)
    eps_b = singles.tile([128, 1], F32)
    with tc.high_priority():
        nc.sync.dma_start(lt_t, logt)
        # it_half = exp(log_inv_t)/2 computed FIRST on scalar: loads the
        # exp table once at t~0 (never again), freeing the finale of it
        nc.gpsimd.memset(bln2, -LN2)
        nc.scalar.activation(it_half, lt_t, AF.Exp, bias=bln2, scale=1.0)
        # pin the sqrt table for the rest of the kernel; every d-chain
        # Sqrt depends on eps_b, so this cannot be scheduled late
        nc.gpsimd.memset(seed, 1e-12)
        # route it_half into seed[0:32]: forces exp (exp table) to run
        # BEFORE this sqrt (sqrt table), so the sqrt table stays resident
        # for the whole kernel after its single load
        nc.gpsimd.tensor_scalar(seed[0:32], it_half, 0.0, 1e-12,
                                op0=ALU.mult, op1=ALU.add)
        nc.scalar.sqrt(eps_b, seed)

    # ---------- rsqrt-with-clamp helper (small tiles only) ----------
    def rsqrt_clamped(dst, ss, pool):
        """dst = 1 / max(sqrt(ss), EPS_NORM), elementwise."""
        shape = list(ss.shape)
        n0 = pool.tile(shape, F32, tag="rsq_n0")
        nc.scalar.sqrt(n0, ss)
        nc.vector.tensor_scalar_max(n0, n0, EPS_NORM)
        nc.vector.reciprocal(dst, n0)

    qss = singles.tile([128, QT], F32)
    W = singles.tile([128, QT * 2 * QT], BF16)   # block-diagonal weights
    nc.gpsimd.memset(W, 0.0)

    with tc.tile_pool(name="rows", bufs=3) as rows, \
         tc.tile_pool(name="sqscr", bufs=2) as sqscr, \
         tc.tile_pool(name="dsq", bufs=2) as dsq, \
         tc.tile_pool(name="smt", bufs=2) as smt, \
         tc.tile_pool(name="ssp", bufs=1, space="PSUM") as ss_pool, \
         tc.tile_pool(name="mm", bufs=3, space="PSUM") as mm_pool, \
         tc.tile_pool(name="mm23", bufs=2, space="PSUM") as mm23_pool, \
         tc.tile_pool(name="maxs", bufs=QT) as maxs_pool:

        maxs_tiles = [None] * QT
        s_tiles = [None] * NCG

        # ---------- d-side chain pieces (per 512-col group) ----------
        def d_square(cg, on_dve=False):
            # dT2 = Square(fp8 dT cols) -> fp8. cg0 runs on the (idle) DVE
            # to shorten the startup chain; the rest on scalar.
            dt2 = dsq.tile([128, KK, 2, 512], F8, tag="dt2")
            if on_dve:
                nc.vector.tensor_tensor(dt2, dT8[:, cg], dT8[:, cg],
                                        op=ALU.mult)
            else:
                nc.scalar.activation(dt2, dT8[:, cg], AF.Square)
            return dt2

        def d_ssmm(cg, dt2):
            # sumsq over h, broadcast over partitions via ones weights
            ssp = ss_pool.tile([128, 512], F32, tag="ss")
            for kk in range(KK):
                nc.tensor.matmul(ssp, ones8, dt2[:, kk],
                                 start=(kk == 0), stop=(kk == KK - 1),
                                 perf_mode=PERF2)
            return ssp

        def d_chain(cg, ssp, split):
            # s = 1 / sqrt(ss + eps) (masked cols were zeroed in the fp8
            # cast, so their sim columns stay 0), then scale the fp8 dT
            # cols in place. split=True fans the 6 column scales across
            # DVE AND gpsimd (latency-critical early groups).
            sq = smt.tile([128, 512], F32, tag="sq")
            nc.scalar.activation(sq, ssp, AF.Sqrt, bias=eps_b)
            sr = smt.tile([128, 512], F32, tag="sr")
            nc.vector.reciprocal_approx_fast(sr, sq)
            sm = smt.tile([128, 512], BF16, tag="sm")
            eng = nc.vector if split else nc.gpsimd
            eng.tensor_copy(sm, sr)
            s_tiles[cg] = sm
            smb = sm[:, None, None, :].broadcast_to((128, KK, 2, 512))
            eng.tensor_tensor(dT8[:, cg], dT8[:, cg], smb, op=ALU.mult)

        # ---------- q row sumsq (scalar Square + free-axis accumulate) ----
        def q_group(c):
            qt_ = rows.tile([128, H], F8, tag="rowtile")
            nc.sync.dma_start(qt_, q_rows[c * 128:(c + 1) * 128, :])
            scr = sqscr.tile([128, H], F8, tag="sq")
            nc.scalar.activation(scr, qt_, AF.Square, accum_out=qss[:, c:c + 1])

        # ---------- main block ----------
        def _lhs(qc, kk):
            return qT8[:, qc // 4, kk, :, (qc % 4) * 128:(qc % 4 + 1) * 128]

        def main_block(qc, cg, reduce_eng):
            if maxs_tiles[qc] is None:
                maxs_tiles[qc] = maxs_pool.tile([128, MLOC], BF16, tag="maxs",
                                                name=f"maxs{qc}")
            ps = mm_pool.tile([128, 512], F32, tag="mm")
            for kk in range(KK):
                nc.tensor.matmul(ps, _lhs(qc, kk), dT8[:, cg, kk],
                                 start=(kk == 0), stop=(kk == KK - 1),
                                 perf_mode=PERF2)
            reduce_eng.reduce_max(
                maxs_tiles[qc][:, 2 * cg:2 * cg + 2],
                ps[:].rearrange("p (d l) -> p d l", l=LD),
                axis=mybir.AxisListType.X)

        def main_block23(qc):
            # cg2+cg3 fused: 6 matmuls into one 2-bank PSUM tile, single
            # batched reduce (fewer DVE ops on the critical tail)
            ps = mm23_pool.tile([128, 1024], F32, tag="mm23")
            for half, cg in enumerate((2, 3)):
                for kk in range(KK):
                    nc.tensor.matmul(ps[:, half * 512:(half + 1) * 512],
                                     _lhs(qc, kk), dT8[:, cg, kk],
                                     start=(kk == 0), stop=(kk == KK - 1),
                                     perf_mode=PERF2)
            nc.vector.reduce_max(
                maxs_tiles[qc][:, 4:8],
                ps[:].rearrange("p (d l) -> p d l", l=LD),
                axis=mybir.AxisListType.X)

        # ---------- cls path (f32, tiny) ----------
        def cls_block():
            qc2 = clsbig.tile([32, H], F32, tag="qc2")
            nc.gpsimd.dma_start(qc2, qcls)
            dcf = clsbig.tile([24, H], F32, tag="dcf")
            nc.gpsimd.dma_start(dcf, dcls)

            qcss = smalls.tile([32, 1], F32, tag="qcss")
            scr1 = clsbig.tile([32, H], F32, tag="clsscr")
            nc.scalar.activation(scr1, qc2, AF.Square, accum_out=qcss)
            dcss = smalls.tile([24, 1], F32, tag="dcss")
            scr2 = clsbig.tile([24, H], F32, tag="clsscr24")
            nc.scalar.activation(scr2, dcf, AF.Square, accum_out=dcss)

            rqc = smalls.tile([32, 1], F32, tag="rqc")
            rsqrt_clamped(rqc, qcss, smalls)
            rdc = smalls.tile([24, 1], F32, tag="rdc")
            rsqrt_clamped(rdc, dcss, smalls)
            nc.scalar.mul(qc2, qc2, rqc)
            nc.scalar.mul(dcf, dcf, rdc)

            KC = H // 128
            qcT = clsbig.tile([128, KC, 32], F32, tag="qcT")
            dcT = clsbig.tile([128, KC, 24], F32, tag="dcT")
            for k in range(KC):
                t1 = mm_pool.tile([128, 32], F32, tag="mm")
                nc.tensor.transpose(t1, qc2[:, k * 128:(k + 1) * 128],
                                    ident[0:32, 0:32])
                nc.scalar.copy(qcT[:, k, :], t1)
                t2 = mm_pool.tile([128, 24], F32, tag="mm")
                nc.tensor.transpose(t2, dcf[:, k * 128:(k + 1) * 128],
                                    ident[0:24, 0:24])
                nc.scalar.copy(dcT[:, k, :], t2)

            cls_ps = mm_pool.tile([32, 24], F32, tag="mm")
            for k in range(KC):
                nc.tensor.matmul(cls_ps, qcT[:, k, :], dcT[:, k, :],
                                 start=(k == 0), stop=(k == KC - 1))

            cls_sb = smalls.tile([32, 24], F32, tag="cls_sb")
            nc.scalar.copy(cls_sb, cls_ps)
            mind = smalls.tile([32, 8], F32, tag="mind")
            nc.vector.tensor_tensor(mind, cls_sb[:, 0:8], cls_sb[:, 8:16],
                                    op=ALU.min)
            wq2 = smalls.tile([32, 8], F32, tag="wq2")  # center - min_doc
            nc.vector.tensor_sub(wq2, cls_sb[:, 16:24], mind)
            return wq2

        def build_W():
            # weighted-sum weights: q_valid / ||q||, scattered block-diagonally
            rq = smalls.tile([128, QT], F32, name="rq")
            rsqrt_clamped(rq, qss, smalls)
            wqw = smalls.tile([128, QT], F32, name="wqw")
            nc.vector.tensor_mul(wqw, qv, rq)
            # W[p, c, j] nonzero at j = 2c + (p>=64): free offset 34c (+1)
            nc.vector.tensor_copy(W[0:64, 0:QT * 2 * QT:2 * QT + 2],
                                  wqw[0:64, :])
            nc.vector.tensor_copy(W[64:128, 1:QT * 2 * QT:2 * QT + 2],
                                  wqw[64:128, :])

        Wv = W[:].rearrange("p (c j) -> p c j", j=2 * QT)

        ws_ref = []

        def ws_mm(j):
            nc.tensor.matmul(ws_ref[0], Wv[:, j, :], maxs_tiles[j],
                             start=(j == 0), stop=(j == QT - 1))

        # ================= emission schedule =================
        # DMA ring order: d-cg0, qT8, d-cg1..3 (one ring, full fan-out).
        # Scalar: exp+sqrt-warm done above; d Squares and Sqrts alternate
        # (both live in the pinned tables, no swaps). cg0/cg1 scale chains
        # run on DVE (latency-critical), cg2/cg3 on gpsimd.
        dt2_0 = d_square(0, on_dve=True)
        ss_0 = d_ssmm(0, dt2_0)
        d_chain(0, ss_0, True)
        dt2_1 = d_square(1)

        for qc in range(0, 8):
            main_block(qc, 0, nc.vector)
        ss_1 = d_ssmm(1, dt2_1)
        d_chain(1, ss_1, False)
        dt2_2 = d_square(2)
        for qc in range(8, QT):
            main_block(qc, 0, nc.vector)
        ss_2 = d_ssmm(2, dt2_2)
        d_chain(2, ss_2, False)
        dt2_3 = d_square(3)

        for qc in range(0, 8):
            main_block(qc, 1, nc.vector)
        ss_3 = d_ssmm(3, dt2_3)
        d_chain(3, ss_3, False)
        for qc in range(8, QT):
            main_block(qc, 1, nc.vector)

        for c in range(QT):
            q_group(c)

        wq2 = cls_block()
        build_W()
        ws_ps = mm_pool.tile([32, MLOC], F32, tag="mm")  # sum_sim accum
        ws_ref.append(ws_ps)

        for qc in range(QT):
            main_block23(qc)
            if qc >= 2:
                ws_mm(qc - 2)
        ws_mm(QT - 2)
        ws_mm(QT - 1)

        # ---------- finale ----------
        avg = smalls.tile([32, 8], F32, tag="avg")
        nc.vector.tensor_scalar(avg, ws_ps, rnv, None, op0=ALU.mult)
        nc.vector.tensor_mul(avg, avg, wq2)
        outt = smalls.tile([32, 8], F32, tag="outt")
        nc.vector.tensor_scalar(outt, avg, it_half, None, op0=ALU.mult)
        nc.sync.dma_start(out, outt)

    ctx.close()


_CACHE = {}


def _build():
    if "nc" in _CACHE:
        return _CACHE["nc"]
    nc = bacc.Bacc("TRN2", target_bir_lowering=False, debug=False,
                   num_devices=NCORES)
    io = {
        "qt8": nc.dram_tensor("qt8", [4 * 128, KK * 2 * 512], F8,
                              kind="ExternalInput"),
        "dt8": nc.dram_tensor("dt8", [NCG * 128, KK * 2 * 512], F8,
                              kind="ExternalInput"),
        "q_rows": nc.dram_tensor("q_rows", [BQ, H], F8,
                                 kind="ExternalInput"),
        "qids": nc.dram_tensor("qids", [B, LQ], I32, kind="ExternalInput"),
        "qids_t": nc.dram_tensor("qids_t", [128, QT], I32,
                                 kind="ExternalInput"),
        "qcls": nc.dram_tensor("qcls", [B, H], F32, kind="ExternalInput"),
        "dcls": nc.dram_tensor("dcls", [L * MLOC, H], F32,
                               kind="ExternalInput"),
        "logt": nc.dram_tensor("logt", [B, 1], F32, kind="ExternalInput"),
        "out": nc.dram_tensor("out", [B, MLOC], F32, kind="ExternalOutput"),
    }
    with tile.TileContext(nc) as tc:
        _emit(nc, tc, io)
    nc.compile()
    _CACHE["nc"] = nc
    return nc


def _dr_layout(xT):
    """(rows, H) f32 -> (128, KK*2*rows) fp8 DoubleRow-transposed layout."""
    rows = xT.shape[0]
    a = xT.reshape(rows, KK, 2, 128)        # [n, kk, i, p]
    a = a.transpose(3, 1, 2, 0)             # [p, kk, i, n]
    return np.ascontiguousarray(a.reshape(128, KK * 2 * rows).astype(NP_F8))


def make_in_maps(q_tok, d_tok, q_cls, d_cls, log_inv_t, q_ids, d_ids,
                 d_punct_mask):
    q_r = np.asarray(q_tok, np.float32).reshape(BQ, H)
    q8 = _dr_layout(q_r)                     # [128, kk*2*BQ]
    q8v = q8.reshape(128, KK, 2, 4, 512).transpose(3, 0, 1, 2, 4)
    qt8 = np.ascontiguousarray(q8v.reshape(4 * 128, KK * 2 * 512))
    q_rows_b = np.ascontiguousarray(q_r.astype(NP_F8))
    qids = np.ascontiguousarray(np.asarray(q_ids, np.int32))
    qids_t = np.ascontiguousarray(qids.reshape(QT, 128).T)
    qcls = np.ascontiguousarray(np.asarray(q_cls, np.float32)[-1])
    logt = np.full((B, 1), np.float32(np.asarray(log_inv_t)), np.float32)
    d_tok = np.asarray(d_tok, np.float32)
    d_cls = np.asarray(d_cls, np.float32)
    d_ids = np.asarray(d_ids, np.int32)
    d_pun = np.asarray(d_punct_mask)
    in_maps = []
    for c in range(NCORES):
        sl = slice(c * MLOC, (c + 1) * MLOC)
        maskcols = ((d_ids[sl].reshape(DR) != 0) & d_pun[sl].reshape(DR))
        d_sh = np.where(maskcols[:, None], d_tok[sl].reshape(DR, H), 0.0)
        d8 = _dr_layout(d_sh)               # [128, kk*2*DR]
        # cg-major reorder: [p, kk, i, n] -> [cg, p, kk, i, 512]
        d8v = d8.reshape(128, KK, 2, NCG, 512).transpose(3, 0, 1, 2, 4)
        dt8 = np.ascontiguousarray(d8v.reshape(NCG * 128, KK * 2 * 512))
        in_maps.append({
            "qt8": qt8,
            "dt8": dt8,
            "q_rows": q_rows_b,
            "qids": qids,
            "qids_t": qids_t,
            "qcls": qcls,
            "dcls": np.ascontiguousarray(d_cls[:, sl, :].reshape(L * MLOC, H)),
            "logt": logt,
        })
    return in_maps


_PERM = np.concatenate([np.arange(0, M, 2), np.arange(1, M, 2)])


def kernel(q_tok, d_tok, q_cls, d_cls, log_inv_t, q_ids, d_ids, d_punct_mask,
           **run_kwargs):
    nc = _build()
    in_maps = make_in_maps(q_tok, d_tok, q_cls, d_cls, log_inv_t, q_ids,
                           d_ids, d_punct_mask)
    res = bass_utils.run_bass_kernel_spmd(nc, in_maps,
                                          core_ids=list(range(NCORES)),
                                          **run_kwargs)
    full = np.concatenate([res.results[c]["out"] for c in range(NCORES)],
                          axis=1)
    out = full[:, _PERM]
    if run_kwargs:
        kernel.last_results = res
    return out
